# revision 4
# baseline (speedup 1.0000x reference)
"""Trainium2 Bass kernel for nn_Net_3762391351854 (GNN alternate-convolution).

Computes, for N=2048 nodes / E=6144 edges:
  node layer:  Hv = (M1 * adj_v) @ (H_v @ W_v) + b_v,  M1 = T diag(d_e) T^T w/ unit diag
  edge layer:  He = norm(M2 * adj_e) @ (H_e @ W_e) + b_e,  M2 = T^T diag(d_v) T w/ unit diag
               norm = divide each column by its max (+1e-10)

Sharding over 8 NeuronCores:
  phase 1: row-shard the [N,N] node adjacency (256 rows/core), computed in
           transposed [j,i] orientation so the follow-up matmul needs no transpose.
           d_v is produced as a fused 65th output column and AllGathered on-device.
  phase 2: column-shard the [E,E] edge adjacency (768 cols/core); colmax is then a
           per-partition free-axis reduce. Normalization is folded into the small
           stationary operand of the final matmul; partial He outputs summed on host.
"""

import os
import sys

import numpy as np

for _p in ("/opt/trn_rl_repo", "/root/.axon_site/_ro/trn_rl_repo"):
    if os.path.isdir(_p) and _p not in sys.path:
        sys.path.append(_p)

import concourse.bass as bass  # noqa: E402
import concourse.tile as tile  # noqa: E402
from concourse import bacc, mybir  # noqa: E402
from concourse.bass_utils import run_bass_kernel_spmd  # noqa: E402

f32 = mybir.dt.float32
bf16 = mybir.dt.bfloat16

N, E = 2048, 6144
NCORES = 8
NI = N // NCORES       # 256 rows of node adjacency per core
EJ = E // NCORES       # 768 cols of edge adjacency per core
NIT = NI // 128        # 2
EJT = EJ // 128        # 6
ET = E // 128          # 48 contraction tiles (phase 1)
KT = N // 128          # 16 contraction tiles (phase 2)
IC = E // 512          # 12 free-dim chunks (phase 2)
JT1 = N // 128         # 16 j-tiles (phase 1)

DEBUG_OUTPUTS = False

_NC_CACHE = {}


def _emit(nc):
    AluOp = mybir.AluOpType
    X = mybir.AxisListType.X

    # ---------------- DRAM I/O ----------------
    TT = nc.dram_tensor("TT", [E, N], f32, kind="ExternalInput")        # T^T (replicated)
    TTi = nc.dram_tensor("TTi", [E, NI], f32, kind="ExternalInput")     # T^T[:, i_chunk]
    AVT = nc.dram_tensor("AVT", [N, NI], f32, kind="ExternalInput")     # adj_v[i_chunk,:]^T
    TF = nc.dram_tensor("TF", [N, E], f32, kind="ExternalInput")        # T (replicated)
    TJ = nc.dram_tensor("TJ", [N, EJ], f32, kind="ExternalInput")       # T[:, j_chunk]
    AET = nc.dram_tensor("AET", [EJ, E], f32, kind="ExternalInput")     # adj_e[:, j_chunk]^T
    HVT = nc.dram_tensor("HVT", [128, N], f32, kind="ExternalInput")    # H_v^T
    WVQ = nc.dram_tensor("WVQ", [128, 65], f32, kind="ExternalInput")   # [W_v | W_v@p_e^T]
    BVC = nc.dram_tensor("BVC", [1, 65], f32, kind="ExternalInput")     # [b_v | b_v@p_e^T]
    HET = nc.dram_tensor("HET", [16, E], f32, kind="ExternalInput")     # H_e^T
    HEJT = nc.dram_tensor("HEJT", [16, EJ], f32, kind="ExternalInput")  # H_e^T[:, j_chunk]
    PVT = nc.dram_tensor("PVT", [16, 1], f32, kind="ExternalInput")     # p_v^T
    WE = nc.dram_tensor("WE", [16, 16], f32, kind="ExternalInput")
    BET = nc.dram_tensor("BET", [1, 16], f32, kind="ExternalInput")     # b_e (core 0) / zeros
    DIDXV = nc.dram_tensor("DIDXV", [128, JT1], f32, kind="ExternalInput")
    DIDXE = nc.dram_tensor("DIDXE", [128, EJT], f32, kind="ExternalInput")
    IOTA2 = nc.dram_tensor("IOTA2", [128, NI], f32, kind="ExternalInput")
    IOTA5 = nc.dram_tensor("IOTA5", [128, 512], f32, kind="ExternalInput")

    HVOUT = nc.dram_tensor("HVOUT", [NI, 64], f32, kind="ExternalOutput")
    HETOUT = nc.dram_tensor("HETOUT", [16, E], f32, kind="ExternalOutput")
    if DEBUG_OUTPUTS:
        DBG_DE = nc.dram_tensor("DBG_DE", [128, ET], f32, kind="ExternalOutput")
        DBG_DV = nc.dram_tensor("DBG_DV", [128, KT], f32, kind="ExternalOutput")
        DBG_CMAX = nc.dram_tensor("DBG_CMAX", [128, EJT], f32, kind="ExternalOutput")

    dv_local = nc.dram_tensor("dv_local", [NIT, 128], f32)
    dv_all = nc.dram_tensor("dv_all", [KT, 128], f32, addr_space="Shared")

    with tile.TileContext(nc) as tc:
        with (
            tc.tile_pool(name="const", bufs=1) as constp,
            tc.tile_pool(name="persist", bufs=1) as perp,
        ):
            # ---- persistent small tiles ----
            iota5 = constp.tile([128, 512], f32)
            nc.sync.dma_start(iota5[:], IOTA5[:])
            ones512 = constp.tile([1, 512], f32)
            nc.gpsimd.memset(ones512[:], 1.0)
            didxe_sb = constp.tile([128, EJT], f32)
            nc.sync.dma_start(didxe_sb[:], DIDXE[:])
            bet_sb = constp.tile([1, 16], f32)
            nc.sync.dma_start(bet_sb[:], BET[:])
            we_sb = constp.tile([16, 16], f32)
            nc.sync.dma_start(we_sb[:], WE[:])
            hejt_sb = constp.tile([16, EJ], f32)
            nc.sync.dma_start(hejt_sb[:], HEJT[:])

            dv_sb = perp.tile([128, NIT], f32)
            dvk_sb = perp.tile([128, KT], f32)
            hwe_sb = perp.tile([128, EJT, 16], f32)
            hwesc_sb = perp.tile([128, EJT, 16], bf16)
            cmax_sb = perp.tile([128, EJT], f32)
            crec_sb = perp.tile([128, EJT], f32)

            # ================= PHASE 1 =================
            with (
                tc.tile_pool(name="p1small", bufs=1) as p1s,
                tc.tile_pool(name="p1tsct", bufs=1) as p1tsctp,
                tc.tile_pool(name="p1tt", bufs=2) as p1tt,
                tc.tile_pool(name="p1avt", bufs=2) as p1avt,
                tc.tile_pool(name="p1adj", bufs=1) as p1adj,
                tc.tile_pool(name="p1mask", bufs=2) as p1mask,
                tc.tile_pool(name="p1hvst", bufs=2) as p1hvst,
                tc.tile_pool(name="ps_pre", bufs=2, space="PSUM") as ps_pre,
                tc.tile_pool(name="ps_m1", bufs=3, space="PSUM") as ps_m1,
                tc.tile_pool(name="ps_hv", bufs=2, space="PSUM") as ps_hv,
            ):
                iota2 = p1s.tile([128, NI], f32)
                nc.sync.dma_start(iota2[:], IOTA2[:])
                ones1 = p1s.tile([1, 128], f32)
                nc.gpsimd.memset(ones1[:], 1.0)
                didxv_sb = p1s.tile([128, JT1], f32)
                nc.sync.dma_start(didxv_sb[:], DIDXV[:])
                wvq_sb = p1s.tile([128, 65], f32)
                nc.sync.dma_start(wvq_sb[:], WVQ[:])
                bvc_sb = p1s.tile([1, 65], f32)
                nc.sync.dma_start(bvc_sb[:], BVC[:])
                het_sb = p1s.tile([16, E], f32)
                nc.sync.dma_start(het_sb[:], HET[:])
                hvt_sb = p1s.tile([128, N], f32)
                nc.sync.dma_start(hvt_sb[:], HVT[:])
                pvt_sb = p1s.tile([16, 1], f32)
                nc.sync.dma_start(pvt_sb[:], PVT[:])
                de_sb = p1s.tile([128, ET], f32)
                rhs65_sb = p1s.tile([128, JT1, 65], f32)
                tsct_sb = p1tsctp.tile([128, ET, NI], f32)

                # HW_e for this core's column chunk (used in phase 2)
                for jt in range(EJT):
                    ps = ps_pre.tile([128, 65], f32, tag="pre")
                    nc.tensor.matmul(
                        ps[:, 0:16], hejt_sb[:, jt * 128:(jt + 1) * 128], we_sb[:],
                        start=True, stop=True,
                    )
                    nc.vector.tensor_copy(hwe_sb[:, jt, :], ps[:, 0:16])

                # d_e = H_e @ p_v^T  (replicated compute)
                for et in range(ET):
                    ps = ps_pre.tile([128, 65], f32, tag="pre")
                    nc.tensor.matmul(
                        ps[:, 0:1], het_sb[:, et * 128:(et + 1) * 128], pvt_sb[:],
                        start=True, stop=True,
                    )
                    nc.vector.tensor_copy(de_sb[:, et:et + 1], ps[:, 0:1])

                # rhs65 = H_v @ [W_v | W_v p_e^T]
                for jt in range(JT1):
                    ps = ps_pre.tile([128, 65], f32, tag="pre")
                    nc.tensor.matmul(
                        ps[:], hvt_sb[:, jt * 128:(jt + 1) * 128], wvq_sb[:],
                        start=True, stop=True,
                    )
                    nc.vector.tensor_copy(rhs65_sb[:, jt, :], ps[:])

                # TscT = T^T[:, i_chunk] * d_e (per-partition scale)
                for et in range(ET):
                    nc.sync.dma_start(
                        tsct_sb[:, et, :], TTi[et * 128:(et + 1) * 128, :]
                    )
                    nc.vector.tensor_scalar_mul(
                        tsct_sb[:, et, :], tsct_sb[:, et, :], de_sb[:, et:et + 1]
                    )

                # mult1^T[j, i_chunk] -> adjusted node adjacency (transposed)
                adjav_sb = p1adj.tile([128, JT1, NI], f32)
                tt_view = TT.ap().rearrange("(a p) j -> p a j", p=128)
                for jt in range(JT1):
                    ttjt = p1tt.tile([128, ET, 128], f32, tag="ttjt")
                    nc.sync.dma_start(
                        ttjt[:], tt_view[:, :, jt * 128:(jt + 1) * 128]
                    )
                    ps = ps_m1.tile([128, NI], f32, tag="m1")
                    for et in range(ET):
                        nc.tensor.matmul(
                            ps[:], ttjt[:, et, :], tsct_sb[:, et, :],
                            start=(et == 0), stop=(et == ET - 1),
                        )
                    avt_t = p1avt.tile([128, NI], f32, tag="avt")
                    nc.sync.dma_start(avt_t[:], AVT[jt * 128:(jt + 1) * 128, :])
                    adj = adjav_sb[:, jt, :]
                    nc.vector.tensor_mul(adj, ps[:], avt_t[:])
                    mask = p1mask.tile([128, NI], mybir.dt.uint8, tag="m")
                    nc.vector.tensor_scalar(
                        mask[:], iota2[:], didxv_sb[:, jt:jt + 1], None,
                        op0=AluOp.is_equal,
                    )
                    nc.vector.copy_predicated(adj, mask[:], avt_t[:])

                # [Hv_out | d_v] = adjAv^T.T @ rhs65 + [b_v | c]
                for it in range(NIT):
                    ps = ps_hv.tile([128, 65], f32, tag="hv")
                    for jt in range(JT1):
                        nc.tensor.matmul(
                            ps[:],
                            adjav_sb[:, jt, it * 128:(it + 1) * 128],
                            rhs65_sb[:, jt, :],
                            start=(jt == 0), stop=False,
                        )
                    nc.tensor.matmul(ps[:], ones1[:], bvc_sb[:], start=False, stop=True)
                    hv_st = p1hvst.tile([128, 64], f32, tag="hvst")
                    nc.vector.tensor_copy(hv_st[:], ps[:, 0:64])
                    nc.sync.dma_start(HVOUT[it * 128:(it + 1) * 128, :], hv_st[:])
                    nc.vector.tensor_copy(dv_sb[:, it:it + 1], ps[:, 64:65])

                # all-gather d_v
                for it in range(NIT):
                    nc.sync.dma_start(dv_local[it, :], dv_sb[:, it])
                nc.gpsimd.collective_compute(
                    "AllGather",
                    AluOp.bypass,
                    replica_groups=[list(range(NCORES))],
                    ins=[dv_local[:]],
                    outs=[dv_all[:]],
                )
                nc.sync.dma_start(dvk_sb[:], dv_all.ap().rearrange("a p -> p a"))

            # ================= PHASE 2 =================
            with (
                tc.tile_pool(name="p2tsc", bufs=1) as p2tscp,
                tc.tile_pool(name="p2b", bufs=1) as p2bp,
                tc.tile_pool(name="p2tic", bufs=18) as p2tic,
                tc.tile_pool(name="p2aet", bufs=3) as p2aet,
                tc.tile_pool(name="p2prod", bufs=2) as p2prod,
                tc.tile_pool(name="p2mask", bufs=2) as p2mask,
                tc.tile_pool(name="p2rm", bufs=2) as p2rm,
                tc.tile_pool(name="p2hest", bufs=2) as p2hest,
                tc.tile_pool(name="ps_m2", bufs=3, space="PSUM") as ps_m2,
                tc.tile_pool(name="ps_he", bufs=2, space="PSUM") as ps_he,
            ):
                tsc2_sb = p2tscp.tile([128, KT, EJ], f32)
                b_sb = p2bp.tile([128, EJT, E], bf16)

                # Tsc2 = T[:, j_chunk] * d_v (per-partition scale)
                for kt in range(KT):
                    nc.sync.dma_start(
                        tsc2_sb[:, kt, :], TJ[kt * 128:(kt + 1) * 128, :]
                    )
                    nc.vector.tensor_scalar_mul(
                        tsc2_sb[:, kt, :], tsc2_sb[:, kt, :], dvk_sb[:, kt:kt + 1]
                    )

                # main sweep: B[j, i] = mult2[j,i] * adj_e[i,j] (+ diag patch), colmax
                for ic in range(IC):
                    tics = []
                    for kt in range(KT):
                        t = p2tic.tile([128, 512], f32, tag="tic")
                        nc.sync.dma_start(
                            t[:], TF[kt * 128:(kt + 1) * 128, ic * 512:(ic + 1) * 512]
                        )
                        tics.append(t)
                    for jt in range(EJT):
                        ps = ps_m2.tile([128, 512], f32, tag="m2")
                        for kt in range(KT):
                            nc.tensor.matmul(
                                ps[:],
                                tsc2_sb[:, kt, jt * 128:(jt + 1) * 128],
                                tics[kt][:],
                                start=(kt == 0), stop=(kt == KT - 1),
                            )
                        aet_t = p2aet.tile([128, 512], f32, tag="aet")
                        nc.sync.dma_start(
                            aet_t[:],
                            AET[jt * 128:(jt + 1) * 128, ic * 512:(ic + 1) * 512],
                        )
                        prod = p2prod.tile([128, 512], f32, tag="prod")
                        nc.vector.tensor_mul(prod[:], ps[:], aet_t[:])
                        mask = p2mask.tile([128, 512], mybir.dt.uint8, tag="mask")
                        nc.vector.tensor_scalar(
                            mask[:], iota5[:], float(ic * 512), didxe_sb[:, jt:jt + 1],
                            op0=AluOp.add, op1=AluOp.is_equal,
                        )
                        nc.vector.copy_predicated(prod[:], mask[:], aet_t[:])
                        rm = p2rm.tile([128, 1], f32, tag="rm")
                        nc.vector.reduce_max(rm[:], prod[:], axis=X)
                        if ic == 0:
                            nc.vector.tensor_copy(cmax_sb[:, jt:jt + 1], rm[:])
                        else:
                            nc.vector.tensor_max(
                                cmax_sb[:, jt:jt + 1], cmax_sb[:, jt:jt + 1], rm[:]
                            )
                        nc.vector.tensor_copy(
                            b_sb[:, jt, ic * 512:(ic + 1) * 512], prod[:]
                        )

                # fold 1/(colmax+eps) into the stationary HW_e operand
                nc.vector.tensor_scalar_add(crec_sb[:], cmax_sb[:], 1e-10)
                nc.vector.reciprocal(crec_sb[:], crec_sb[:])
                for jt in range(EJT):
                    nc.vector.tensor_scalar_mul(
                        hwesc_sb[:, jt, :], hwe_sb[:, jt, :], crec_sb[:, jt:jt + 1]
                    )

                # He^T partial = (recip*HW_e)^T @ B (+ b_e via ones, core 0 only)
                for ic in range(IC):
                    ps = ps_he.tile([16, 512], f32, tag="he")
                    for jt in range(EJT):
                        nc.tensor.matmul(
                            ps[:],
                            hwesc_sb[:, jt, :],
                            b_sb[:, jt, ic * 512:(ic + 1) * 512],
                            start=(jt == 0), stop=False,
                        )
                    nc.tensor.matmul(
                        ps[:], bet_sb[:], ones512[:], start=False, stop=True
                    )
                    he_st = p2hest.tile([16, 512], f32, tag="hest")
                    nc.vector.tensor_copy(he_st[:], ps[:])
                    nc.sync.dma_start(HETOUT[:, ic * 512:(ic + 1) * 512], he_st[:])

                if DEBUG_OUTPUTS:
                    nc.sync.dma_start(DBG_DE[:], de_sb[:])
                    nc.sync.dma_start(DBG_DV[:], dvk_sb[:])
                    nc.sync.dma_start(DBG_CMAX[:], cmax_sb[:])
    return nc


def _get_nc():
    if "nc" not in _NC_CACHE:
        nc = bacc.Bacc(
            "TRN2", target_bir_lowering=False, debug=False, num_devices=NCORES
        )
        _emit(nc)
        nc.compile()
        _NC_CACHE["nc"] = nc
    return _NC_CACHE["nc"]


def kernel(**inputs):
    H_v = np.ascontiguousarray(np.asarray(inputs["H_v"], dtype=np.float32))
    H_e = np.ascontiguousarray(np.asarray(inputs["H_e"], dtype=np.float32))
    adj_v = np.ascontiguousarray(np.asarray(inputs["adj_v"], dtype=np.float32))
    adj_e = np.ascontiguousarray(np.asarray(inputs["adj_e"], dtype=np.float32))
    T = np.ascontiguousarray(np.asarray(inputs["T"], dtype=np.float32))
    W_v = np.asarray(inputs["W_v"], dtype=np.float32)
    b_v = np.asarray(inputs["b_v"], dtype=np.float32)
    p_v = np.asarray(inputs["p_v"], dtype=np.float32)
    W_e = np.asarray(inputs["W_e"], dtype=np.float32)
    b_e = np.asarray(inputs["b_e"], dtype=np.float32)
    p_e = np.asarray(inputs["p_e"], dtype=np.float32)

    TT = np.ascontiguousarray(T.T)                       # [E, N]
    HVT = np.ascontiguousarray(H_v.T)                    # [128, N]
    HET = np.ascontiguousarray(H_e.T)                    # [16, E]
    WVQ = np.ascontiguousarray(
        np.concatenate([W_v, W_v @ p_e.T], axis=1), dtype=np.float32
    )                                                    # [128, 65]
    c_scalar = float(b_v @ p_e[0])
    BVC = np.concatenate(
        [b_v[None, :], np.array([[c_scalar]], dtype=np.float32)], axis=1
    ).astype(np.float32)                                 # [1, 65]
    PVT = np.ascontiguousarray(p_v.T)                    # [16, 1]
    iota2 = np.broadcast_to(
        np.arange(NI, dtype=np.float32)[None, :], (128, NI)
    ).copy()
    iota5 = np.broadcast_to(
        np.arange(512, dtype=np.float32)[None, :], (128, 512)
    ).copy()
    prow = np.arange(128, dtype=np.float32)

    in_maps = []
    for c in range(NCORES):
        isl = slice(c * NI, (c + 1) * NI)
        jsl = slice(c * EJ, (c + 1) * EJ)
        didxv = np.empty((128, JT1), dtype=np.float32)
        for jt in range(JT1):
            didxv[:, jt] = (jt * 128 + prow) - c * NI
        didxe = np.empty((128, EJT), dtype=np.float32)
        for jt in range(EJT):
            didxe[:, jt] = (c * EJT + jt) * 128 + prow
        in_maps.append({
            "TT": TT,
            "TTi": np.ascontiguousarray(TT[:, isl]),
            "AVT": np.ascontiguousarray(adj_v[isl, :].T),
            "TF": T,
            "TJ": np.ascontiguousarray(T[:, jsl]),
            "AET": np.ascontiguousarray(adj_e[:, jsl].T),
            "HVT": HVT,
            "WVQ": WVQ,
            "BVC": BVC,
            "HET": HET,
            "HEJT": np.ascontiguousarray(HET[:, jsl]),
            "PVT": PVT,
            "WE": W_e,
            "BET": (b_e[None, :] if c == 0
                    else np.zeros((1, 16), dtype=np.float32)),
            "DIDXV": didxv,
            "DIDXE": didxe,
            "IOTA2": iota2,
            "IOTA5": iota5,
        })

    nc = _get_nc()
    trace_kw = {}
    if os.environ.get("BASS_KERNEL_TRACE"):
        trace_kw = {"trace": True, "trace_cores": list(range(NCORES))}
    res = run_bass_kernel_spmd(nc, in_maps, core_ids=list(range(NCORES)), **trace_kw)
    kernel._last_results = res

    hv = np.concatenate(
        [res.results[c]["HVOUT"] for c in range(NCORES)], axis=0
    ).astype(np.float32)                                 # [N, 64]
    he = np.zeros((16, E), dtype=np.float32)
    for c in range(NCORES):
        he += res.results[c]["HETOUT"]
    he = np.ascontiguousarray(he.T, dtype=np.float32)    # [E, 16]
    return hv, he


# revision 5
# speedup vs baseline: 2.6128x; 2.6128x over previous
"""Trainium2 Bass kernel for nn_Net_3762391351854 (GNN alternate-convolution).

Computes, for N=2048 nodes / E=6144 edges:
  node layer:  Hv = (M1 * adj_v) @ (H_v @ W_v) + b_v,  M1 = T diag(d_e) T^T w/ unit diag
  edge layer:  He = norm(M2 * adj_e) @ (H_e @ W_e) + b_e,  M2 = T^T diag(d_v) T w/ unit diag
               norm = divide each column by its max (+1e-10)

Sharding over 8 NeuronCores:
  phase 1: row-shard the [N,N] node adjacency (256 rows/core), computed in
           transposed [j,i] orientation so the follow-up matmul needs no transpose.
           d_v is produced as a fused 65th output column and AllGathered on-device.
  phase 2: column-shard the [E,E] edge adjacency (768 cols/core); colmax is then a
           per-partition free-axis reduce. Normalization is folded into the small
           stationary operand of the final matmul; partial He outputs summed on host.

The two big T-contractions run in fp16 (full 16-bit PE rate; fp32 matmul is
decomposed 2x and streams ~3x slower). d_v is pre-scaled by 1/8192 so the fp16
operands/results stay well inside range; the column normalization cancels the
scale exactly (up to the 1e-10 epsilon, which is ~1e-12 relative here).
"""

import os
import sys

import numpy as np

for _p in ("/opt/trn_rl_repo", "/root/.axon_site/_ro/trn_rl_repo"):
    if os.path.isdir(_p) and _p not in sys.path:
        sys.path.append(_p)

import concourse.bass as bass  # noqa: E402
import concourse.tile as tile  # noqa: E402
from concourse import bacc, mybir  # noqa: E402
from concourse.bass_utils import run_bass_kernel_spmd  # noqa: E402

f32 = mybir.dt.float32
f16 = mybir.dt.float16

N, E = 2048, 6144
NCORES = 8
NI = N // NCORES       # 256 rows of node adjacency per core
EJ = E // NCORES       # 768 cols of edge adjacency per core
NIT = NI // 128        # 2
EJT = EJ // 128        # 6
ET = E // 128          # 48 contraction tiles (phase 1)
KT = N // 128          # 16 contraction tiles (phase 2)
IC = E // 512          # 12 free-dim chunks (phase 2)
JT1 = N // 128         # 16 j-tiles (phase 1)
S = 8192.0             # d_v pre-scale so fp16 phase-2 values stay in range

_NC_CACHE = {}


def _emit(nc):
    AluOp = mybir.AluOpType
    X = mybir.AxisListType.X

    # ---------------- DRAM I/O ----------------
    TT = nc.dram_tensor("TT", [E, N], f16, kind="ExternalInput")        # T^T (replicated)
    TTi = nc.dram_tensor("TTi", [E, NI], f16, kind="ExternalInput")     # T^T[:, i_chunk]
    AVT = nc.dram_tensor("AVT", [N, NI], f32, kind="ExternalInput")     # adj_v[i_chunk,:]^T
    TF = nc.dram_tensor("TF", [N, E], f16, kind="ExternalInput")        # T (replicated)
    TJ = nc.dram_tensor("TJ", [N, EJ], f16, kind="ExternalInput")       # T[:, j_chunk]
    AET = nc.dram_tensor("AET", [EJ, E], f32, kind="ExternalInput")     # adj_e[:, j_chunk]^T
    ADIAG = nc.dram_tensor("ADIAG", [128, EJT], f32, kind="ExternalInput")  # diag(adj_e)/S
    HVT = nc.dram_tensor("HVT", [128, N], f32, kind="ExternalInput")    # H_v^T
    WVQ = nc.dram_tensor("WVQ", [128, 65], f32, kind="ExternalInput")   # [W_v | W_v@p_e^T]
    BVC = nc.dram_tensor("BVC", [1, 65], f32, kind="ExternalInput")     # [b_v | b_v@p_e^T]
    HET = nc.dram_tensor("HET", [16, E], f32, kind="ExternalInput")     # H_e^T
    HEJT = nc.dram_tensor("HEJT", [16, EJ], f32, kind="ExternalInput")  # H_e^T[:, j_chunk]
    PVT = nc.dram_tensor("PVT", [16, 1], f32, kind="ExternalInput")     # p_v^T
    WE = nc.dram_tensor("WE", [16, 16], f32, kind="ExternalInput")
    BET = nc.dram_tensor("BET", [1, 16], f32, kind="ExternalInput")     # b_e (core 0) / zeros
    DIDXV = nc.dram_tensor("DIDXV", [128, JT1], f32, kind="ExternalInput")
    DIDXE = nc.dram_tensor("DIDXE", [128, EJT], f32, kind="ExternalInput")
    IOTA2 = nc.dram_tensor("IOTA2", [128, NI], f32, kind="ExternalInput")
    IOTA5 = nc.dram_tensor("IOTA5", [128, 512], f32, kind="ExternalInput")

    HVOUT = nc.dram_tensor("HVOUT", [NI, 64], f32, kind="ExternalOutput")
    HETOUT = nc.dram_tensor("HETOUT", [16, E], f32, kind="ExternalOutput")

    dv_local = nc.dram_tensor("dv_local", [NIT, 128], f32)
    dv_all = nc.dram_tensor("dv_all", [KT, 128], f32, addr_space="Shared")

    with tile.TileContext(nc) as tc:
        with (
            tc.tile_pool(name="const", bufs=1) as constp,
            tc.tile_pool(name="persist", bufs=1) as perp,
        ):
            # ---- persistent small tiles ----
            iota5 = constp.tile([128, 512], f32)
            nc.sync.dma_start(iota5[:], IOTA5[:])
            ones512 = constp.tile([1, 512], f32)
            nc.gpsimd.memset(ones512[:], 1.0)
            didxe_sb = constp.tile([128, EJT], f32)
            nc.sync.dma_start(didxe_sb[:], DIDXE[:])
            adiag_sb = constp.tile([128, EJT], f32)
            nc.sync.dma_start(adiag_sb[:], ADIAG[:])
            bet_sb = constp.tile([1, 16], f32)
            nc.sync.dma_start(bet_sb[:], BET[:])
            we_sb = constp.tile([16, 16], f32)
            nc.sync.dma_start(we_sb[:], WE[:])
            hejt_sb = constp.tile([16, EJ], f32)
            nc.sync.dma_start(hejt_sb[:], HEJT[:])

            dv_sb = perp.tile([128, NIT], f32)
            dvk_sb = perp.tile([128, KT], f32)
            dvs_sb = perp.tile([128, KT], f32)
            hwe_sb = perp.tile([128, EJT, 16], f32)
            hwesc_sb = perp.tile([128, EJT, 16], f16)
            cmax_sb = perp.tile([128, EJT], f32)
            crec_sb = perp.tile([128, EJT], f32)

            # ================= PHASE 1 =================
            with (
                tc.tile_pool(name="p1small", bufs=1) as p1s,
                tc.tile_pool(name="p1tsct", bufs=1) as p1tsctp,
                tc.tile_pool(name="p1tt", bufs=3) as p1tt,
                tc.tile_pool(name="p1avt", bufs=2) as p1avt,
                tc.tile_pool(name="p1adj", bufs=1) as p1adj,
                tc.tile_pool(name="p1mask", bufs=2) as p1mask,
                tc.tile_pool(name="p1hvst", bufs=2) as p1hvst,
                tc.tile_pool(name="ps_pre", bufs=2, space="PSUM") as ps_pre,
                tc.tile_pool(name="ps_m1", bufs=3, space="PSUM") as ps_m1,
                tc.tile_pool(name="ps_hv", bufs=2, space="PSUM") as ps_hv,
            ):
                iota2 = p1s.tile([128, NI], f32)
                nc.sync.dma_start(iota2[:], IOTA2[:])
                ones1 = p1s.tile([1, 128], f32)
                nc.gpsimd.memset(ones1[:], 1.0)
                didxv_sb = p1s.tile([128, JT1], f32)
                nc.sync.dma_start(didxv_sb[:], DIDXV[:])
                wvq_sb = p1s.tile([128, 65], f32)
                nc.sync.dma_start(wvq_sb[:], WVQ[:])
                bvc_sb = p1s.tile([1, 65], f32)
                nc.sync.dma_start(bvc_sb[:], BVC[:])
                het_sb = p1s.tile([16, E], f32)
                nc.sync.dma_start(het_sb[:], HET[:])
                hvt_sb = p1s.tile([128, N], f32)
                nc.sync.dma_start(hvt_sb[:], HVT[:])
                pvt_sb = p1s.tile([16, 1], f32)
                nc.sync.dma_start(pvt_sb[:], PVT[:])
                de_sb = p1s.tile([128, ET], f32)
                rhs65_sb = p1s.tile([128, JT1, 65], f32)
                tsct_sb = p1tsctp.tile([128, ET, NI], f16)

                # HW_e for this core's column chunk (used in phase 2)
                for jt in range(EJT):
                    ps = ps_pre.tile([128, 65], f32, tag="pre")
                    nc.tensor.matmul(
                        ps[:, 0:16], hejt_sb[:, jt * 128:(jt + 1) * 128], we_sb[:],
                        start=True, stop=True,
                    )
                    nc.vector.tensor_copy(hwe_sb[:, jt, :], ps[:, 0:16])

                # d_e = H_e @ p_v^T  (replicated compute)
                for et in range(ET):
                    ps = ps_pre.tile([128, 65], f32, tag="pre")
                    nc.tensor.matmul(
                        ps[:, 0:1], het_sb[:, et * 128:(et + 1) * 128], pvt_sb[:],
                        start=True, stop=True,
                    )
                    nc.vector.tensor_copy(de_sb[:, et:et + 1], ps[:, 0:1])

                # rhs65 = H_v @ [W_v | W_v p_e^T]
                for jt in range(JT1):
                    ps = ps_pre.tile([128, 65], f32, tag="pre")
                    nc.tensor.matmul(
                        ps[:], hvt_sb[:, jt * 128:(jt + 1) * 128], wvq_sb[:],
                        start=True, stop=True,
                    )
                    nc.vector.tensor_copy(rhs65_sb[:, jt, :], ps[:])

                # TscT = T^T[:, i_chunk] * d_e (per-partition scale), fp16
                for et in range(ET):
                    nc.sync.dma_start(
                        tsct_sb[:, et, :], TTi[et * 128:(et + 1) * 128, :]
                    )
                    nc.vector.tensor_scalar_mul(
                        tsct_sb[:, et, :], tsct_sb[:, et, :], de_sb[:, et:et + 1]
                    )

                # mult1^T[j, i_chunk] -> adjusted node adjacency (transposed)
                adjav_sb = p1adj.tile([128, JT1, NI], f32)
                tt_view = TT.ap().rearrange("(a p) j -> p a j", p=128)
                for jt in range(JT1):
                    ttjt = p1tt.tile([128, ET, 128], f16, tag="ttjt")
                    nc.sync.dma_start(
                        ttjt[:], tt_view[:, :, jt * 128:(jt + 1) * 128]
                    )
                    ps = ps_m1.tile([128, NI], f32, tag="m1")
                    for et in range(ET):
                        nc.tensor.matmul(
                            ps[:], ttjt[:, et, :], tsct_sb[:, et, :],
                            start=(et == 0), stop=(et == ET - 1),
                        )
                    avt_t = p1avt.tile([128, NI], f32, tag="avt")
                    nc.sync.dma_start(avt_t[:], AVT[jt * 128:(jt + 1) * 128, :])
                    adj = adjav_sb[:, jt, :]
                    nc.vector.tensor_mul(adj, ps[:], avt_t[:])
                    mask = p1mask.tile([128, NI], mybir.dt.uint8, tag="m")
                    nc.vector.tensor_scalar(
                        mask[:], iota2[:], didxv_sb[:, jt:jt + 1], None,
                        op0=AluOp.is_equal,
                    )
                    nc.vector.copy_predicated(adj, mask[:], avt_t[:])

                # [Hv_out | d_v] = adjAv^T.T @ rhs65 + [b_v | c]
                for it in range(NIT):
                    ps = ps_hv.tile([128, 65], f32, tag="hv")
                    for jt in range(JT1):
                        nc.tensor.matmul(
                            ps[:],
                            adjav_sb[:, jt, it * 128:(it + 1) * 128],
                            rhs65_sb[:, jt, :],
                            start=(jt == 0), stop=False,
                        )
                    nc.tensor.matmul(ps[:], ones1[:], bvc_sb[:], start=False, stop=True)
                    hv_st = p1hvst.tile([128, 64], f32, tag="hvst")
                    nc.vector.tensor_copy(hv_st[:], ps[:, 0:64])
                    nc.sync.dma_start(HVOUT[it * 128:(it + 1) * 128, :], hv_st[:])
                    nc.vector.tensor_copy(dv_sb[:, it:it + 1], ps[:, 64:65])

                # all-gather d_v
                for it in range(NIT):
                    nc.sync.dma_start(dv_local[it, :], dv_sb[:, it])
                nc.gpsimd.collective_compute(
                    "AllGather",
                    AluOp.bypass,
                    replica_groups=[list(range(NCORES))],
                    ins=[dv_local[:]],
                    outs=[dv_all[:]],
                )
                nc.sync.dma_start(dvk_sb[:], dv_all.ap().rearrange("a p -> p a"))
                nc.vector.tensor_scalar_mul(dvs_sb[:], dvk_sb[:], 1.0 / S)

            # ================= PHASE 2 =================
            with (
                tc.tile_pool(name="p2tsc", bufs=1) as p2tscp,
                tc.tile_pool(name="p2b", bufs=1) as p2bp,
                tc.tile_pool(name="p2tic", bufs=18) as p2tic,
                tc.tile_pool(name="p2aet", bufs=3) as p2aet,
                tc.tile_pool(name="p2prod", bufs=3) as p2prod,
                tc.tile_pool(name="p2mask", bufs=2) as p2mask,
                tc.tile_pool(name="p2rm", bufs=2) as p2rm,
                tc.tile_pool(name="p2hest", bufs=2) as p2hest,
                tc.tile_pool(name="ps_m2", bufs=4, space="PSUM") as ps_m2,
                tc.tile_pool(name="ps_he", bufs=2, space="PSUM") as ps_he,
            ):
                tsc2_sb = p2tscp.tile([128, KT, EJ], f16)
                b_sb = p2bp.tile([128, EJT, E], f16)

                # Tsc2 = T[:, j_chunk] * (d_v/S) (per-partition scale), fp16
                for kt in range(KT):
                    nc.sync.dma_start(
                        tsc2_sb[:, kt, :], TJ[kt * 128:(kt + 1) * 128, :]
                    )
                    nc.vector.tensor_scalar_mul(
                        tsc2_sb[:, kt, :], tsc2_sb[:, kt, :], dvs_sb[:, kt:kt + 1]
                    )

                # main sweep: B[j, i] = (mult2[j,i]/S) * adj_e[i,j] (+ diag), colmax
                for ic in range(IC):
                    tics = []
                    for kt in range(KT):
                        t = p2tic.tile([128, 512], f16, tag="tic")
                        nc.sync.dma_start(
                            t[:], TF[kt * 128:(kt + 1) * 128, ic * 512:(ic + 1) * 512]
                        )
                        tics.append(t)
                    for jt in range(EJT):
                        ps = ps_m2.tile([128, 512], f32, tag="m2")
                        for kt in range(KT):
                            nc.tensor.matmul(
                                ps[:],
                                tsc2_sb[:, kt, jt * 128:(jt + 1) * 128],
                                tics[kt][:],
                                start=(kt == 0), stop=(kt == KT - 1),
                            )
                        aet_t = p2aet.tile([128, 512], f32, tag="aet")
                        nc.sync.dma_start(
                            aet_t[:],
                            AET[jt * 128:(jt + 1) * 128, ic * 512:(ic + 1) * 512],
                        )
                        prod = p2prod.tile([128, 512], f32, tag="prod")
                        nc.vector.tensor_mul(prod[:], ps[:], aet_t[:])
                        mask = p2mask.tile([128, 512], mybir.dt.uint8, tag="mask")
                        nc.vector.tensor_scalar(
                            mask[:], iota5[:], float(ic * 512), didxe_sb[:, jt:jt + 1],
                            op0=AluOp.add, op1=AluOp.is_equal,
                        )
                        nc.vector.copy_predicated(
                            prod[:], mask[:],
                            adiag_sb[:, jt, None].to_broadcast([128, 512]),
                        )
                        rm = p2rm.tile([128, 1], f32, tag="rm")
                        nc.vector.reduce_max(rm[:], prod[:], axis=X)
                        if ic == 0:
                            nc.vector.tensor_copy(cmax_sb[:, jt:jt + 1], rm[:])
                        else:
                            nc.vector.tensor_max(
                                cmax_sb[:, jt:jt + 1], cmax_sb[:, jt:jt + 1], rm[:]
                            )
                        nc.vector.tensor_copy(
                            b_sb[:, jt, ic * 512:(ic + 1) * 512], prod[:]
                        )

                # fold 1/(colmax+eps) into the stationary HW_e operand
                nc.vector.tensor_scalar_add(crec_sb[:], cmax_sb[:], 1e-10)
                nc.vector.reciprocal(crec_sb[:], crec_sb[:])
                for jt in range(EJT):
                    nc.vector.tensor_scalar_mul(
                        hwesc_sb[:, jt, :], hwe_sb[:, jt, :], crec_sb[:, jt:jt + 1]
                    )

                # He^T partial = (recip*HW_e)^T @ B (+ b_e via ones, core 0 only)
                for ic in range(IC):
                    ps = ps_he.tile([16, 512], f32, tag="he")
                    for jt in range(EJT):
                        nc.tensor.matmul(
                            ps[:],
                            hwesc_sb[:, jt, :],
                            b_sb[:, jt, ic * 512:(ic + 1) * 512],
                            start=(jt == 0), stop=False,
                        )
                    nc.tensor.matmul(
                        ps[:], bet_sb[:], ones512[:], start=False, stop=True
                    )
                    he_st = p2hest.tile([16, 512], f32, tag="hest")
                    nc.vector.tensor_copy(he_st[:], ps[:])
                    nc.sync.dma_start(HETOUT[:, ic * 512:(ic + 1) * 512], he_st[:])
    return nc


def _get_nc():
    if "nc" not in _NC_CACHE:
        nc = bacc.Bacc(
            "TRN2", target_bir_lowering=False, debug=False, num_devices=NCORES
        )
        _emit(nc)
        nc.compile()
        _NC_CACHE["nc"] = nc
    return _NC_CACHE["nc"]


def kernel(**inputs):
    H_v = np.ascontiguousarray(np.asarray(inputs["H_v"], dtype=np.float32))
    H_e = np.ascontiguousarray(np.asarray(inputs["H_e"], dtype=np.float32))
    adj_v = np.ascontiguousarray(np.asarray(inputs["adj_v"], dtype=np.float32))
    adj_e = np.ascontiguousarray(np.asarray(inputs["adj_e"], dtype=np.float32))
    T = np.ascontiguousarray(np.asarray(inputs["T"], dtype=np.float32))
    W_v = np.asarray(inputs["W_v"], dtype=np.float32)
    b_v = np.asarray(inputs["b_v"], dtype=np.float32)
    p_v = np.asarray(inputs["p_v"], dtype=np.float32)
    W_e = np.asarray(inputs["W_e"], dtype=np.float32)
    b_e = np.asarray(inputs["b_e"], dtype=np.float32)
    p_e = np.asarray(inputs["p_e"], dtype=np.float32)

    T16 = T.astype(np.float16)
    TT16 = np.ascontiguousarray(T16.T)                   # [E, N] fp16
    HVT = np.ascontiguousarray(H_v.T)                    # [128, N]
    HET = np.ascontiguousarray(H_e.T)                    # [16, E]
    WVQ = np.ascontiguousarray(
        np.concatenate([W_v, W_v @ p_e.T], axis=1), dtype=np.float32
    )                                                    # [128, 65]
    c_scalar = float(b_v @ p_e[0])
    BVC = np.concatenate(
        [b_v[None, :], np.array([[c_scalar]], dtype=np.float32)], axis=1
    ).astype(np.float32)                                 # [1, 65]
    PVT = np.ascontiguousarray(p_v.T)                    # [16, 1]
    iota2 = np.broadcast_to(
        np.arange(NI, dtype=np.float32)[None, :], (128, NI)
    ).copy()
    iota5 = np.broadcast_to(
        np.arange(512, dtype=np.float32)[None, :], (128, 512)
    ).copy()
    prow = np.arange(128, dtype=np.float32)
    adiag_full = np.diagonal(adj_e).astype(np.float32) / np.float32(S)  # [E]

    in_maps = []
    for c in range(NCORES):
        isl = slice(c * NI, (c + 1) * NI)
        jsl = slice(c * EJ, (c + 1) * EJ)
        didxv = np.empty((128, JT1), dtype=np.float32)
        for jt in range(JT1):
            didxv[:, jt] = (jt * 128 + prow) - c * NI
        didxe = np.empty((128, EJT), dtype=np.float32)
        for jt in range(EJT):
            didxe[:, jt] = (c * EJT + jt) * 128 + prow
        adiag = np.ascontiguousarray(
            adiag_full[jsl].reshape(EJT, 128).T
        )                                                # [128, EJT]
        in_maps.append({
            "TT": TT16,
            "TTi": np.ascontiguousarray(TT16[:, isl]),
            "AVT": np.ascontiguousarray(adj_v[isl, :].T),
            "TF": T16,
            "TJ": np.ascontiguousarray(T16[:, jsl]),
            "AET": np.ascontiguousarray(adj_e[:, jsl].T),
            "ADIAG": adiag,
            "HVT": HVT,
            "WVQ": WVQ,
            "BVC": BVC,
            "HET": HET,
            "HEJT": np.ascontiguousarray(HET[:, jsl]),
            "PVT": PVT,
            "WE": W_e,
            "BET": (b_e[None, :] if c == 0
                    else np.zeros((1, 16), dtype=np.float32)),
            "DIDXV": didxv,
            "DIDXE": didxe,
            "IOTA2": iota2,
            "IOTA5": iota5,
        })

    nc = _get_nc()
    trace_kw = {}
    if os.environ.get("BASS_KERNEL_TRACE"):
        trace_kw = {"trace": True, "trace_cores": list(range(NCORES))}
    res = run_bass_kernel_spmd(nc, in_maps, core_ids=list(range(NCORES)), **trace_kw)
    kernel._last_results = res

    hv = np.concatenate(
        [res.results[c]["HVOUT"] for c in range(NCORES)], axis=0
    ).astype(np.float32)                                 # [N, 64]
    he = np.zeros((16, E), dtype=np.float32)
    for c in range(NCORES):
        he += res.results[c]["HETOUT"]
    he = np.ascontiguousarray(he.T, dtype=np.float32)    # [E, 16]
    return hv, he
